# revision 28
# baseline (speedup 1.0000x reference)
"""Trainium2 Bass kernel for nn_CapsAll: r-head structured self-attention +
capsule votes + dynamic routing, data-parallel over batch across 8 cores.

Math (per sample b):
  hbar  = relu(x @ WS1[r].T)            [T, DA]   per head r
  score = hbar @ WS2[r].T               [T, U2]
  alpha = softmax(score over T)
  m     = sum_t alpha * x               [U2]
  votes = m @ capsule_weights[r]        [SC*OA]
  3x dynamic routing over (r, SC, OA) -> class logits [SC]

v3 design (per core, 16 samples), tuned against real-HW measurements
(LDWEIGHTS amortization and ACT-accumulator reductions matter far more on
hardware than the cost model suggests):
  - The two big matmuls (hbar, scores) run in fp8e4m3 with
    perf_mode=DoubleRow (0.5 cycles/row): contraction in K=256 chunks,
    operands laid out [128, k-pair, ...].
  - x is shipped from host in both fp8 (matmul) and bf16 (weighted sum).
  - Stage-1/2 matmuls run over 3-pair sample blocks so each LDWEIGHTS is
    reused 3x (per-MM weight reloads serialize badly on real HW).
  - exp runs on ACT, bf16 out; ALL softmax denominators Z via accum_out.
  - alpha*x product out-of-place in bf16, split GPSIMD/DVE halves.
  - m_num reduction split: 4 samples via ACT Copy+accum, 12 via DVE reduce.
  - relu->fp8 PSUM evacuation split ACT/DVE 1:3 (GPSIMD cannot read PSUM).
  - All DMA issued from the SP (sync) engine queues.
  - votes matmul in bf16; all 4 SC*OA chunks accumulate in one PSUM bank at
    partition offsets 0/32/64/96 via col tile_position.
  - routing identical to v1 (fp32/fp32r on [16r+b] x [c,o] layout).
"""
import numpy as np
import ml_dtypes

import concourse.bass as bass
import concourse.tile as tile
from concourse import bacc, mybir
from concourse.bass_utils import run_bass_kernel_spmd

F32 = mybir.dt.float32
F32R = mybir.dt.float32r
BF16 = mybir.dt.bfloat16
F8 = mybir.dt.float8e4
AF = mybir.ActivationFunctionType
ALU = mybir.AluOpType
AX = mybir.AxisListType
DR = mybir.MatmulPerfMode.DoubleRow

R = 8
U2 = 1024
DA = 512
SC = 128
OA = 16
NR = 3
B = 128
T = 256
NCORES = 8
BLOC = B // NCORES          # 16 samples per core
NPAIR = BLOC // 2           # 8 sample-pairs (512 cols per pair)
UC = U2 // 128              # 8 u-chunks
AC = DA // 128              # 4 a-chunks
OC4 = 4                     # SC*OA = 2048 -> 4 chunks of 512
ACT_Z_PAIRS = 8
MNUM_ACT_N = 4
MUL_POOL_UCS = set(range(8))              # pairs 0..5: Z via ACT accum; 6..7: DVE reduce


def build_bass(loops: int = 1, stage: str = "full", mul_mode: str = "split",
               expf: str = "exp", act_z_pairs: int = ACT_Z_PAIRS,
               skips: frozenset = frozenset(), oop_mul: bool = True,
               mnum_mode: str = "split"):
    nc = bacc.Bacc("TRN2", target_bir_lowering=False)

    x8_d = nc.declare_dram_parameter("x8", [128, UC, BLOC, T], F8, isOutput=False)
    x16_d = nc.declare_dram_parameter("x16", [128, UC, BLOC, T], BF16, isOutput=False)
    w1_d = nc.declare_dram_parameter("w1", [R, 128, UC, AC, 128], F8, isOutput=False)
    w2_d = nc.declare_dram_parameter("w2", [R, 128, AC, UC, 128], F8, isOutput=False)
    cw_d = nc.declare_dram_parameter("cw", [R, UC, 128, SC * OA], BF16, isOutput=False)
    sm_d = nc.declare_dram_parameter("smask", [128, BLOC], F32R, isOutput=False)
    p2_d = nc.declare_dram_parameter("p2", [BLOC, 128], F32R, isOutput=False)
    out_d = nc.declare_dram_parameter("out", [BLOC, SC], F32, isOutput=True)

    with tile.TileContext(nc) as tc:
        with (
            tc.tile_pool(name="consts", bufs=1) as consts,
            tc.tile_pool(name="xres", bufs=1) as xres,
            tc.tile_pool(name="wpool", bufs=2) as wpool,
            tc.tile_pool(name="cwpool", bufs=2) as cwpool,
            tc.tile_pool(name="hpool", bufs=1) as hpool,
            tc.tile_pool(name="epool", bufs=2) as epool,
            tc.tile_pool(name="mpool", bufs=1) as mpool,
            tc.tile_pool(name="m16pool", bufs=3) as m16pool,
            tc.tile_pool(name="smallpool", bufs=8) as smallpool,
            tc.tile_pool(name="vpool", bufs=1) as vpool,
            tc.tile_pool(name="rpool", bufs=1) as rpool,
        ):
            smask_sb = consts.tile([128, BLOC], F32R)
            nc.sync.dma_start(smask_sb[:], sm_d.ap())
            p2_sb = consts.tile([BLOC, 128], F32R)
            nc.sync.dma_start(p2_sb[:], p2_d.ap())

            # resident x in both precisions: [128(up), uc, smp, t]
            x8_sb = xres.tile([128, UC, BLOC, T], F8)
            x16_sb = xres.tile([128, UC, BLOC, T], BF16)
            # startup loads spread across four DMA queues so the
            # single-shot (loops=1) path doesn't serialize on one engine
            engs = [nc.sync, nc.scalar, nc.gpsimd, nc.sync]
            for h in range(2):
                sl = slice(h * (UC // 2), (h + 1) * (UC // 2))
                engs[2 * h].dma_start(x8_sb[:, sl, :, :], x8_d.ap()[:, sl])
                engs[2 * h + 1].dma_start(x16_sb[:, sl, :, :], x16_d.ap()[:, sl])

            def one_pass():
                votes_pack = mpool.tile([128, OC4, 512], F32, tag="votes_pack")

                m16s = [None, None]
                last_expd = [None]

                with (
                    tc.tile_pool(name="s1_psum", bufs=1, space="PSUM") as s1_psum,
                    tc.tile_pool(name="s2_psum", bufs=4, space="PSUM") as s2_psum,
                    tc.tile_pool(name="vt_psum", bufs=1, space="PSUM") as vt_psum,
                ):
                    def emit_exp(r, uc, p, ps_s, expd, z_sb):
                        if expf == "copy":
                            nc.scalar.copy(expd[:, 2 * p:2 * p + 2, :], ps_s[:])
                        elif p < act_z_pairs and stage != "mm":
                            for s2 in range(2):
                                nc.scalar.activation(
                                    expd[:, 2 * p + s2, :],
                                    ps_s[:, s2, :],
                                    AF.Exp,
                                    accum_out=z_sb[:, 2 * p + s2:2 * p + s2 + 1],
                                )
                        else:
                            nc.scalar.activation(
                                expd[:, 2 * p:2 * p + 2, :], ps_s[:], AF.Exp,
                            )

                    def emit_votes(rv):
                        # votes: [16, 2048] = m16.T @ CW[rv], bf16; 4
                        # oc-chunks share one PSUM bank at col offsets
                        m16v = m16s[rv % 2]
                        ps_v = vt_psum.tile([128, 512], F32, tag="ps_v",
                                            name=f"ps_v_{rv}")
                        for uc in range(UC):
                            cw_t = cwpool.tile([128, SC * OA], BF16, tag="cw")
                            nc.sync.dma_start(cw_t[:], cw_d.ap()[rv, uc])
                            for oc in range(OC4):
                                nc.tensor.matmul(
                                    ps_v[32 * oc:32 * oc + BLOC, :],
                                    m16v[:, uc, :],
                                    cw_t[:, oc * 512:(oc + 1) * 512],
                                    start=(uc == 0),
                                    stop=(uc == UC - 1),
                                    tile_position=(0, 32 * oc),
                                )
                        vstage = vpool.tile([BLOC, OC4, 512], F32, tag="vstage")
                        for oc in range(OC4):
                            nc.scalar.copy(
                                vstage[:, oc, :],
                                ps_v[32 * oc:32 * oc + BLOC, :],
                            )
                        nc.sync.dma_start(
                            votes_pack[16 * rv:16 * (rv + 1), :, :], vstage[:]
                        )

                    for r in range(R):
                        w1_sb = wpool.tile([128, UC, AC, 128], F8, tag="w1")
                        nc.sync.dma_start(w1_sb[:], w1_d.ap()[r])
                        w2_sb = wpool.tile([128, AC, UC, 128], F8, tag="w2")
                        nc.sync.dma_start(w2_sb[:], w2_d.ap()[r])

                        # ---- stage 1: hbar = relu(x @ W1^T) in fp8 ----
                        # hbar [128(ap), ac, smp, t] fp8
                        hbar = hpool.tile([128, AC, BLOC, T], F8, tag="hbar")
                        for pgrp in ((0, 1, 2), (3, 4, 5), (6, 7)):
                            for ac in range(AC):
                                ps_h = {
                                    p: s1_psum.tile([128, 512], F32,
                                                    tag=f"h{j}",
                                                    name=f"h{j}_{r}_{ac}_{p}")
                                    for j, p in enumerate(pgrp)
                                }
                                for kp in range(UC // 2):
                                    for p in pgrp:
                                        nc.tensor.matmul(
                                            ps_h[p][:],
                                            w1_sb[:, 2 * kp:2 * kp + 2, ac, :],
                                            x8_sb[:, 2 * kp:2 * kp + 2,
                                                  2 * p:2 * p + 2, :],
                                            start=(kp == 0),
                                            stop=(kp == UC // 2 - 1),
                                            perf_mode=DR,
                                        )
                                for p in pgrp:
                                    if p % 4 == 0:
                                        nc.scalar.activation(
                                            hbar[:, ac, 2 * p:2 * p + 2, :],
                                            ps_h[p][:], AF.Relu,
                                        )
                                    else:
                                        nc.vector.tensor_scalar(
                                            out=hbar[:, ac, 2 * p:2 * p + 2, :],
                                            in0=ps_h[p][:],
                                            scalar1=0.0, scalar2=None,
                                            op0=ALU.max,
                                        )

                        # ---- votes for the PREVIOUS head (keeps PE off
                        # the exp critical path) ----
                        if r > 0 and stage == "full":
                            emit_votes(r - 1)

                        # ---- stage 2: scores -> exp -> m ----
                        m16 = m16pool.tile([128, UC, BLOC], BF16, tag="m16",
                                           name=f"m16_{r}")
                        m16s[r % 2] = m16
                        if stage == "mm":
                            nc.vector.memset(m16[:], 0.0)
                        for uc in range(UC):
                            expd = epool.tile([128, BLOC, T], BF16,
                                              tag="expd", name=f"expd_{r}_{uc}")
                            last_expd[0] = expd
                            z_sb = smallpool.tile([128, BLOC], F32, tag="z_sb",
                                              name=f"z_{r}_{uc}")
                            for pblk in ((0, 1, 2, 3), (4, 5, 6, 7)):
                                ps_blk = {
                                    p: s2_psum.tile([128, 2, T], F32, tag="ps_s",
                                                    name=f"s_{r}_{uc}_{p}")
                                    for p in pblk
                                }
                                # kp outer so the stationary W2 slice is
                                # reused across the pair block (LDWEIGHTS
                                # amortization on real hardware)
                                for kp in range(AC // 2):
                                    for p in pblk:
                                        nc.tensor.matmul(
                                            ps_blk[p][:].rearrange("p a b -> p (a b)"),
                                            w2_sb[:, 2 * kp:2 * kp + 2, uc, :],
                                            hbar[:, 2 * kp:2 * kp + 2,
                                                 2 * p:2 * p + 2, :],
                                            start=(kp == 0),
                                            stop=(kp == AC // 2 - 1),
                                            perf_mode=DR,
                                        )
                                for p in pblk:
                                    emit_exp(r, uc, p, ps_blk[p], expd, z_sb)

                            if stage == "mm":
                                continue
                            nAZ = 2 * act_z_pairs
                            if nAZ < BLOC and "zres" not in skips:
                                nc.vector.reduce_sum(
                                    out=z_sb[:, nAZ:], in_=expd[:, nAZ:, :],
                                    axis=AX.X,
                                )
                            elif nAZ < BLOC:
                                nc.vector.memset(z_sb[:, nAZ:], 1.0)
                            # prod = expd * x (in-place, bf16 2x on DVE;
                            # a share goes to gpsimd which can't touch PSUM
                            # but runs SBUF tensor_tensor fine)
                            mul_out = expd
                            if oop_mul:
                                prod = epool.tile([128, BLOC, T], BF16,
                                                  tag="prod", name=f"prod_{r}_{uc}")
                                mul_out = prod
                            if "mul" in skips:
                                mul_out = expd
                            elif mul_mode == "pool":
                                nc.gpsimd.tensor_tensor(
                                    out=mul_out[:], in0=expd[:],
                                    in1=x16_sb[:, uc, :, :], op=ALU.mult,
                                )
                            elif mul_mode == "dve":
                                nc.vector.tensor_tensor(
                                    out=mul_out[:], in0=expd[:],
                                    in1=x16_sb[:, uc, :, :], op=ALU.mult,
                                )
                            else:  # split halves across Pool and DVE
                                nc.gpsimd.tensor_tensor(
                                    out=mul_out[:, :8, :], in0=expd[:, :8, :],
                                    in1=x16_sb[:, uc, :8, :], op=ALU.mult,
                                )
                                nc.vector.tensor_tensor(
                                    out=mul_out[:, 8:, :], in0=expd[:, 8:, :],
                                    in1=x16_sb[:, uc, 8:, :], op=ALU.mult,
                                )
                            m_num = smallpool.tile([128, BLOC], F32, tag="m_num",
                                               name=f"mn_{r}_{uc}")
                            if "mnum" in skips:
                                nc.vector.memset(m_num[:], 1.0)
                            elif mnum_mode == "act":
                                # per-sample Copy+accum on ACT: the scratch
                                # full-size out overwrites expd (dead after
                                # the mul); accum_out delivers the t-sum
                                for s in range(BLOC):
                                    nc.scalar.activation(
                                        expd[:, s, :], mul_out[:, s, :],
                                        AF.Copy,
                                        accum_out=m_num[:, s:s + 1],
                                    )
                            elif mnum_mode == "split":
                                nact = MNUM_ACT_N
                                for s in range(nact):
                                    nc.scalar.activation(
                                        expd[:, s, :], mul_out[:, s, :],
                                        AF.Copy,
                                        accum_out=m_num[:, s:s + 1],
                                    )
                                nc.vector.reduce_sum(
                                    out=m_num[:, nact:],
                                    in_=mul_out[:, nact:, :],
                                    axis=AX.X,
                                )
                            else:
                                nc.vector.reduce_sum(
                                    out=m_num[:], in_=mul_out[:], axis=AX.X,
                                )
                            zr = smallpool.tile([128, BLOC], F32, tag="zr",
                                            name=f"zr_{r}_{uc}")
                            nc.vector.reciprocal(zr[:], z_sb[:])
                            nc.vector.tensor_tensor(
                                out=m16[:, uc, :], in0=m_num[:], in1=zr[:],
                                op=ALU.mult,
                            )

                    if stage == "full":
                        emit_votes(R - 1)

                # ---- dynamic routing on [p' = 16r+b] x [c, o] ----
                votes_v = votes_pack[:].rearrange("p a (c o) -> p a c o", o=OA)
                logits = rpool.tile([128, SC], F32, tag="logits")
                nc.vector.memset(logits[:], 0.0)
                out_sb = rpool.tile([BLOC, SC], F32, tag="out_sb")
                if stage != "full":
                    # keep a data dep on the m pipeline so it isn't dead code
                    nc.vector.memset(out_sb[:], 0.0)
                    m16l = m16s[(R - 1) % 2]
                    nc.vector.tensor_tensor(out=out_sb[:, :UC],
                                            in0=m16l[:BLOC, :, 0],
                                            in1=out_sb[:, :UC], op=ALU.add)
                    if last_expd[0] is not None:
                        nc.vector.tensor_tensor(out=out_sb[:, :8],
                                                in0=last_expd[0][:BLOC, 0, :8],
                                                in1=out_sb[:, :8], op=ALU.add)
                    nc.sync.dma_start(out_d.ap(), out_sb[:])
                    return

                with (
                    tc.tile_pool(name="rt_psum", bufs=1, space="PSUM") as rt_psum,
                    tc.tile_pool(name="ab_psum", bufs=2, space="PSUM") as ab_psum,
                ):
                    for k in range(NR):
                        route_e = rpool.tile([128, SC], F32, tag="route_e")
                        rs = rpool.tile([128, 1], F32, tag="rs")
                        nc.scalar.activation(route_e[:], logits[:], AF.Exp,
                                             accum_out=rs[:])
                        rr = rpool.tile([128, 1], F32, tag="rr")
                        nc.vector.reciprocal(rr[:], rs[:])
                        route = rpool.tile([128, SC], F32, tag="route")
                        nc.vector.tensor_scalar_mul(route[:], route_e[:], rr[:])
                        ps_p = [rt_psum.tile([BLOC, 512], F32, tag=f"ps_p{oc}",
                                             name=f"ps_p{oc}_{k}")
                                for oc in range(OC4)]
                        for oc in range(OC4):
                            route_b = (route[:, oc * 32:(oc + 1) * 32]
                                       .unsqueeze(-1).to_broadcast([128, 32, OA]))
                            tmp = rpool.tile([128, 32, OA], F32R,
                                             tag=f"tmp{oc % 2}",
                                             name=f"tmp_{k}_{oc}")
                            nc.vector.tensor_tensor(
                                out=tmp[:], in0=votes_v[:, oc, :, :],
                                in1=route_b, op=ALU.mult
                            )
                            nc.tensor.matmul(
                                ps_p[oc][:],
                                smask_sb[:],
                                tmp[:].rearrange("p c o -> p (c o)"),
                                start=True,
                                stop=True,
                            )
                        n2 = rpool.tile([BLOC, SC], F32, tag="n2")
                        for oc in range(OC4):
                            sqs = rpool.tile([BLOC, 512], F32, tag="scr512")
                            nc.scalar.square(sqs[:], ps_p[oc][:])
                            nc.vector.reduce_sum(
                                out=n2[:, oc * 32:(oc + 1) * 32],
                                in_=sqs[:].rearrange("p (c o) -> p c o", o=OA),
                                axis=AX.X,
                            )
                        n2p1 = rpool.tile([BLOC, SC], F32, tag="n2p1")
                        nc.vector.tensor_scalar_add(n2p1[:], n2[:], 1.0)
                        r2 = rpool.tile([BLOC, SC], F32, tag="r2")
                        nc.vector.reciprocal(r2[:], n2p1[:])
                        if k == NR - 1:
                            nc.vector.tensor_tensor(
                                out=out_sb[:], in0=n2[:], in1=r2[:], op=ALU.mult
                            )
                            break
                        sq2 = rpool.tile([BLOC, SC], F32, tag="sq2")
                        nc.scalar.sqrt(sq2[:], n2[:])
                        scale = rpool.tile([BLOC, SC], F32, tag="scale")
                        nc.vector.tensor_tensor(
                            out=scale[:], in0=sq2[:], in1=r2[:], op=ALU.mult
                        )
                        act_sb = rpool.tile([BLOC, OC4, 32, OA], F32R, tag="act_sb")
                        for oc in range(OC4):
                            scale_b = (scale[:, oc * 32:(oc + 1) * 32]
                                       .unsqueeze(-1).to_broadcast([BLOC, 32, OA]))
                            nc.vector.tensor_tensor(
                                out=act_sb[:, oc, :, :],
                                in0=ps_p[oc][:].rearrange("p (c o) -> p c o", o=OA),
                                in1=scale_b,
                                op=ALU.mult,
                            )
                        dist = rpool.tile([128, SC], F32, tag="dist")
                        for oc in range(OC4):
                            ps_a = ab_psum.tile([128, 512], F32, tag="ps_a")
                            nc.tensor.matmul(
                                ps_a[:],
                                p2_sb[:],
                                act_sb[:, oc, :, :].rearrange("p c o -> p (c o)"),
                                start=True,
                                stop=True,
                            )
                            dtmp = rpool.tile([128, 512], F32, tag="scr512")
                            nc.vector.tensor_tensor(
                                out=dtmp[:],
                                in0=votes_pack[:, oc, :],
                                in1=ps_a[:],
                                op=ALU.mult,
                            )
                            nc.vector.reduce_sum(
                                out=dist[:, oc * 32:(oc + 1) * 32],
                                in_=dtmp[:].rearrange("p (c o) -> p c o", o=OA),
                                axis=AX.X,
                            )
                        nc.vector.tensor_tensor(
                            out=logits[:], in0=logits[:], in1=dist[:], op=ALU.add
                        )
                nc.sync.dma_start(out_d.ap(), out_sb[:])

            if loops == 1:
                one_pass()
            else:
                with tc.For_i(0, loops, 1):
                    one_pass()

    nc.compile()
    return nc


def prep_inputs(x, WS1, WS2, capsule_weights):
    """Host-side reshapes/casts into the device layouts (numpy)."""
    E4 = ml_dtypes.float8_e4m3
    BF = ml_dtypes.bfloat16
    x = np.ascontiguousarray(x, dtype=np.float32)
    # [B, T, U2] -> per core [16, T, U2] -> [up, uc, smp, t]
    xs = x.reshape(NCORES, BLOC, T, UC, 128)
    xs = xs.transpose(0, 4, 3, 1, 2)  # [core, up, uc, smp, t]
    xs8 = np.ascontiguousarray(xs.astype(E4))
    xs16 = np.ascontiguousarray(xs.astype(BF))

    w1 = np.ascontiguousarray(WS1, dtype=np.float32)  # [R, DA, U2]
    w1 = w1.reshape(R, AC, 128, UC, 128)              # [r, ac, ap, uc, up]
    w1 = np.ascontiguousarray(w1.transpose(0, 4, 3, 1, 2).astype(E4))

    w2 = np.ascontiguousarray(WS2, dtype=np.float32)  # [R, U2, DA]
    w2 = w2.reshape(R, UC, 128, AC, 128)              # [r, uc, up, ac, ap]
    w2 = np.ascontiguousarray(w2.transpose(0, 4, 3, 1, 2).astype(E4))

    cw = np.ascontiguousarray(capsule_weights, dtype=np.float32)
    cw = np.ascontiguousarray(cw.reshape(R, UC, 128, SC * OA).astype(BF))

    pidx = np.arange(128)
    smask = (pidx[:, None] % BLOC == np.arange(BLOC)[None, :]).astype(np.float32)
    p2 = np.ascontiguousarray(smask.T)

    shared = {"w1": w1, "w2": w2, "cw": cw, "smask": smask, "p2": p2}
    in_maps = [{"x8": xs8[c], "x16": xs16[c], **shared} for c in range(NCORES)]
    return in_maps


_NC_CACHE = {}


def kernel(x, WS1, WS2, capsule_weights):
    in_maps = prep_inputs(np.asarray(x), np.asarray(WS1), np.asarray(WS2),
                          np.asarray(capsule_weights))
    if "nc" not in _NC_CACHE:
        _NC_CACHE["nc"] = build_bass(loops=1)
    nc = _NC_CACHE["nc"]
    res = run_bass_kernel_spmd(nc, in_maps, list(range(NCORES)))
    out = np.concatenate([res.results[c]["out"] for c in range(NCORES)], axis=0)
    return out.astype(np.float32)


if __name__ == "__main__":
    import reference

    inputs = {k: np.asarray(v) for k, v in reference.setup_inputs().items()}
    expected = np.asarray(reference.reference(**inputs))
    got = kernel(**inputs)
    err = np.abs(got - expected)
    denom = np.abs(expected).max()
    print("max abs err:", err.max(), "rel:", err.max() / denom)
